# revision 48
# baseline (speedup 1.0000x reference)
"""Trainium2 Bass kernel for nn_EdgeLayer (gnn_message_passing).

out[e] = g(neighbors[e]) where g[v] = (MLP(edge_features[v]).reshape(16,16))
@ node_features[v]: only the 50k per-node values are distinct. Nodes are
split contiguously across the 8 cores (6250 each); edges sorted by
neighbor on the host follow their node's core.

Single fused launch per core:
  - MLP over the core's 6250 nodes in 6 PE passes: L1/L2/L3 (relu
    PSUM->SBUF copies alternating Scalar/DVE), W4 split into two 128-row
    halves whose rows are reordered (p = 8i+j) so both halves share ONE
    selector matmul; b4 folded into Scalar Identity+bias copies; the
    per-node einsum uses DVE Hadamard products with host-built nf
    replications.
  - The selector matmul is widened to 128 output partitions, so the
    node table lands in SBUF already replicated 8x (partition p holds
    feature p%16): tab[p, v] = g[p%16, v], stored bf16.
  - Output: two dense [128, V] bf16 DRAM writes (rep0/rep1) give 16
    replica slots per node -- contiguous DMA, no per-edge descriptors.
    Edges with per-node rank >= 16 (max degree ~27, ~400 edges/core)
    are served by one on-chip ap_gather (GpSimd, d=2 pair mode) into a
    [128, 256] residual tile; the host picks the correct pair half.
Host-side work is index bookkeeping + a bijective permutation of
device-written slots into edge order.
"""
import numpy as np
import ml_dtypes

import concourse.bass as bass
import concourse.tile as tile
from concourse import ap_utils, bacc, mybir
from concourse import bass_utils

E = 500000
N = 50000
D_IN = 32
D_HID = 128
D_NODE = 16
N_CORES = 8
V_NODE = N // N_CORES            # 6250 nodes per core
V_CORE = 6272                    # padded node capacity per core
CHUNKS = [256, 256] + [512] * 11 + [128]
NCH = len(CHUNKS)
R_MAIN = 16                      # dense replica slots per node
NCH_EARLY = 13                   # chunks whose rep DMAs go out inline
DUMMY_GATHER = True
S_LAG = 9
W4_LAG = 6
CAST_DVE_ALL = False
NRES_G = 128                     # residual ap_gather slots per 16-part group
NRES = NRES_G * 8                # residual slots per core

BF = ml_dtypes.bfloat16
TRACE = False
last_exec_ns = {"mlp": None, "gather": None}

_cache = {}


def _build_fused():
    f32 = mybir.dt.float32
    bf16 = mybir.dt.bfloat16
    i16 = mybir.dt.int16
    nc = bacc.Bacc("TRN2", target_bir_lowering=False, debug=False,
                   num_devices=N_CORES)
    efT = nc.dram_tensor("efT", [D_HID, V_CORE], bf16,
                         kind="ExternalInput").ap()
    nfa = nc.dram_tensor("nfa", [128, V_CORE], bf16, kind="ExternalInput").ap()
    nfb = nc.dram_tensor("nfb", [128, V_CORE], bf16, kind="ExternalInput").ap()
    w1 = nc.dram_tensor("w1", [D_HID, D_HID], bf16, kind="ExternalInput").ap()
    # packed [w2 | w3 | w4a | w4b | swd] as [128, 640] bf16
    wpk = nc.dram_tensor("wpk", [D_HID, 5 * D_HID], bf16,
                         kind="ExternalInput").ap()
    # packed [b1 b2 b3 b4a b4b] as [128, 5] f32
    bpk = nc.dram_tensor("bpk", [D_HID, 5], f32, kind="ExternalInput").ap()
    ridx = nc.dram_tensor("ridx", [128, NRES_G // 16], i16,
                          kind="ExternalInput").ap()
    rep0 = nc.dram_tensor("rep0", [128, V_CORE], bf16, kind="ExternalOutput").ap()
    rep1 = nc.dram_tensor("rep1", [128, V_CORE], bf16, kind="ExternalOutput").ap()
    res = nc.dram_tensor("res", [128, 2 * NRES_G], bf16,
                         kind="ExternalOutput").ap()

    Relu = mybir.ActivationFunctionType.Relu
    Ident = mybir.ActivationFunctionType.Identity
    AluAdd = mybir.AluOpType.add
    AluMax = mybir.AluOpType.max
    with tile.TileContext(nc) as tc:
        with (
            tc.tile_pool(name="const", bufs=1) as cpool,
            tc.tile_pool(name="big", bufs=1) as bpool,
            tc.tile_pool(name="sm", bufs=4) as spool,
            tc.tile_pool(name="psX", bufs=3, space="PSUM") as psX,
            tc.tile_pool(name="psY", bufs=3, space="PSUM") as psY,
            tc.tile_pool(name="psZ", bufs=2, space="PSUM") as psZ,
        ):
            w1t = cpool.tile([D_HID, D_HID], bf16)
            nc.sync.dma_start(w1t[:], w1[:])
            eft = bpool.tile([D_HID, V_CORE], bf16, tag="eft")
            offs = [0]
            for w in CHUNKS:
                offs.append(offs[-1] + w)
            # eft loads split across the Act and SP DMA queues, in
            # groups: issue rate, not transfer, limits L1
            nc.scalar.dma_start(eft[:, offs[0] : offs[2]],
                                efT[:, offs[0] : offs[2]])
            bt = cpool.tile([D_HID, 5], f32)
            nc.scalar.dma_start(bt[:], bpk[:])
            nc.scalar.dma_start(eft[:, offs[2] : offs[5]],
                                efT[:, offs[2] : offs[5]])
            wpkt = cpool.tile([D_HID, 5 * D_HID], bf16)
            nc.sync.dma_start(wpkt[:], wpk[:])
            for k0, k1 in ((5, 9), (9, 12), (12, 14)):
                sl = slice(offs[k0], offs[k1])
                nc.sync.dma_start(eft[:, sl], efT[:, sl])
            ridxt = cpool.tile([128, NRES_G // 16], i16)
            nc.sync.dma_start(ridxt[:], ridx[:])
            w2t = wpkt[:, 0:128]
            w3t = wpkt[:, 128:256]
            w4at = wpkt[:, 256:384]
            w4bt = wpkt[:, 384:512]
            swdt = wpkt[:, 512:640]

            # nf replications load as interleaved halves so the first
            # Hadamard chunks aren't gated on the full 3.4MB transfer
            H = V_CORE // 2
            nfat = bpool.tile([128, V_CORE], bf16, tag="nfat")
            nfbt = bpool.tile([128, V_CORE], bf16, tag="nfbt")
            nc.sync.dma_start(nfat[:, :H], nfa[:, :H])
            nc.sync.dma_start(nfbt[:, :H], nfb[:, :H])
            nc.sync.dma_start(nfat[:, H:], nfa[:, H:])
            nc.sync.dma_start(nfbt[:, H:], nfb[:, H:])
            hA = bpool.tile([D_HID, V_CORE], bf16, tag="hA")
            hB = bpool.tile([D_HID, V_CORE], bf16, tag="hB")
            hC = bpool.tile([D_HID, V_CORE], bf16, tag="hC")
            tab = bpool.tile([128, V_CORE], bf16, tag="tab")

            # Scalar warmup: the first activation triggers a ~1.3us
            # ACT_TABLE_LOAD; fire it during startup, off the fill path.
            dsc = cpool.tile([128, 1], f32)
            nc.vector.memset(dsc[:], 0.0)
            dso = cpool.tile([128, 1], f32)
            nc.scalar.activation(dso[:], dsc[:], Relu)

            if DUMMY_GATHER:
                # Early ap_gather warms the GPSIMD library while DMAs
                # are few.
                din = cpool.tile([128, 16], f32)
                nc.gpsimd.memset(din[:], 0.0)
                didx = cpool.tile([128, 1], i16)
                nc.gpsimd.memset(didx[:], 0)
                dout = cpool.tile([128, 16], f32)
                nc.gpsimd.ap_gather(
                    dout[:].rearrange("p (n d) -> p n d", d=1),
                    din[:].rearrange("p (n d) -> p n d", d=1),
                    didx[:], channels=128, num_elems=16, d=1, num_idxs=16,
                )

            # Single 6-stage software pipeline over chunks: L1, L2(lag 2),
            # L3(lag 4), w4a/w4b(lag 6), selector(lag 9). Consecutive PE
            # matmuls always differ in weights and keep the 128x128 shape,
            # engines see a blended relu + tail load throughout, and the
            # pipeline fills only once.
            def relu_copy(k, bcol, dst_t, p, w, sl):
                if (k + bcol) % 2 == 1:
                    nc.scalar.activation(dst_t[:, sl], p[:, :w], Relu,
                                         bias=bt[:, bcol : bcol + 1])
                else:
                    nc.vector.tensor_scalar(dst_t[:, sl], p[:, :w],
                                            bt[:, bcol : bcol + 1], 0.0,
                                            AluAdd, AluMax)

            qqs = {}
            for j in range(NCH + S_LAG + 1):
                # L1
                if j < NCH:
                    k, w = j, CHUNKS[j]
                    sl = slice(offs[k], offs[k] + w)
                    p = psX.tile([D_HID, 512], mybir.dt.float32, tag="p")
                    nc.tensor.matmul(p[:, :w], w1t[:], eft[:, sl],
                                     start=True, stop=True)
                    relu_copy(k, 0, hA, p, w, sl)
                # L2
                k = j - 2
                if 0 <= k < NCH:
                    w = CHUNKS[k]
                    sl = slice(offs[k], offs[k] + w)
                    p = psY.tile([D_HID, 512], mybir.dt.float32, tag="p")
                    nc.tensor.matmul(p[:, :w], w2t, hA[:, sl],
                                     start=True, stop=True)
                    relu_copy(k, 1, hB, p, w, sl)
                # L3
                k = j - 4
                if 0 <= k < NCH:
                    w = CHUNKS[k]
                    sl = slice(offs[k], offs[k] + w)
                    p = psZ.tile([D_HID, 512], mybir.dt.float32, tag="p")
                    nc.tensor.matmul(p[:, :w], w3t, hB[:, sl],
                                     start=True, stop=True)
                    relu_copy(k, 2, hC, p, w, sl)
                # w4 halves + Hadamard
                k = j - W4_LAG
                if 0 <= k < NCH:
                    w = CHUNKS[k]
                    sl = slice(offs[k], offs[k] + w)
                    pa = psX.tile([D_HID, 512], mybir.dt.float32, tag="p")
                    nc.tensor.matmul(pa[:, :w], w4at, hC[:, sl],
                                     start=True, stop=True)
                    pb = psY.tile([D_HID, 512], mybir.dt.float32, tag="p")
                    nc.tensor.matmul(pb[:, :w], w4bt, hC[:, sl],
                                     start=True, stop=True)
                    paS = spool.tile([D_HID, 512], bf16, tag="paS")
                    nc.scalar.activation(paS[:, :w], pa[:, :w], Ident,
                                         bias=bt[:, 3:4])
                    pbS = spool.tile([D_HID, 512], bf16, tag="pbS")
                    nc.scalar.activation(pbS[:, :w], pb[:, :w], Ident,
                                         bias=bt[:, 4:5])
                    tA = spool.tile([D_HID, 512], bf16, tag="tA")
                    nc.vector.tensor_mul(tA[:, :w], paS[:, :w], nfat[:, sl])
                    tB = spool.tile([D_HID, 512], bf16, tag="tB")
                    nc.vector.tensor_mul(tB[:, :w], pbS[:, :w], nfbt[:, sl])
                    qq = spool.tile([D_HID, 512], bf16, tag="qq")
                    nc.vector.tensor_add(qq[:, :w], tA[:, :w], tB[:, :w])
                    qqs[k] = qq
                # selector -> tab -> rep DMAs
                k = j - S_LAG
                if 0 <= k < NCH:
                    w = CHUNKS[k]
                    sl = slice(offs[k], offs[k] + w)
                    qq = qqs.pop(k)
                    pg = psZ.tile([128, 512], mybir.dt.float32, tag="p")
                    nc.tensor.matmul(pg[:, :w], swdt, qq[:, :w],
                                     start=True, stop=True)
                    if CAST_DVE_ALL or k % 2 == 0:
                        nc.vector.tensor_copy(tab[:, sl], pg[:, :w])
                    else:
                        nc.scalar.activation(tab[:, sl], pg[:, :w], Ident)
                    # rep writes batched every 2 chunks (halves DMA issues)
                    if k % 2 == 1 and k < NCH_EARLY:
                        bsl = slice(offs[k - 1], offs[k] + w)
                        nc.sync.dma_start(rep0[:, bsl], tab[:, bsl])
                        nc.sync.dma_start(rep1[:, bsl], tab[:, bsl])


            # Residual edges (per-node rank >= R_MAIN): on-chip pair
            # gather from the bf16 table; host picks the half.
            rest = bpool.tile([128, 2 * NRES_G], bf16, tag="rest")
            nc.gpsimd.ap_gather(
                rest[:].rearrange("p (n d) -> p n d", d=2),
                tab[:].rearrange("p (n d) -> p n d", d=2),
                ridxt[:],
                channels=128, num_elems=V_CORE // 2, d=2, num_idxs=NRES_G,
            )
            nc.scalar.dma_start(res[:], rest[:])
            late = slice(offs[NCH_EARLY - 1], V_CORE)
            nc.sync.dma_start(rep0[:, late], tab[:, late])
            nc.sync.dma_start(rep1[:, late], tab[:, late])
    nc.compile()
    return nc


def kernel(**inputs):
    ef = np.asarray(inputs["edge_features"], dtype=np.float32)
    nf = np.asarray(inputs["node_features"], dtype=np.float32)
    ei = np.asarray(inputs["edge_index"])
    Ws = [np.asarray(inputs[k], dtype=np.float32) for k in ("W1", "W2", "W3", "W4")]
    bs = [np.asarray(inputs[k], dtype=np.float32) for k in ("b1", "b2", "b3", "b4")]

    if "fused" not in _cache:
        _cache["fused"] = _build_fused()

    # ---- host index bookkeeping: sort edges by neighbor ----
    nb = ei[:, 1].astype(np.int64)
    order = np.argsort(nb, kind="stable")
    snb = nb[order]
    splits = np.searchsorted(snb, np.arange(1, N_CORES) * V_NODE)
    bounds = [0] + list(splits) + [E]

    # shared weight-derived inputs
    p128 = np.arange(128)
    idxA = 16 * (p128 // 8) + (p128 % 8)
    idxB = idxA + 8
    swd_np = (p128[:, None] // 8 == p128[None, :] % 16).astype(np.float32)
    wpk_np = np.concatenate([
        Ws[1].T, Ws[2].T, Ws[3][idxA].T, Ws[3][idxB].T, swd_np], axis=1)
    bpk_np = np.stack([bs[0], bs[1], bs[2], bs[3][idxA], bs[3][idxB]], axis=1)
    shared = {
        "w1": np.ascontiguousarray(
            np.pad(Ws[0].T, ((0, D_HID - D_IN), (0, 0))).astype(BF)),
        "wpk": np.ascontiguousarray(wpk_np.astype(BF)),
        "bpk": np.ascontiguousarray(bpk_np),
    }

    ef_pad = np.zeros((N + V_CORE, D_HID), np.float32)
    ef_pad[:N, :D_IN] = ef[:N]
    nf_pad = np.zeros((N + V_CORE, D_NODE), np.float32)
    nf_pad[:N] = nf[:N]

    ins = []
    meta = []
    for c in range(N_CORES):
        seg = snb[bounds[c] : bounds[c + 1]]
        lo = c * V_NODE
        ec = len(seg)
        vloc = (seg - lo).astype(np.int64)
        first = np.searchsorted(seg, seg, side="left")
        rank = np.arange(ec) - first
        resid = rank >= R_MAIN
        nres = int(resid.sum())
        assert nres <= NRES, f"core {c}: {nres} residual edges > {NRES}"

        nfc = nf_pad[lo : lo + V_CORE]                    # [V, 16]
        nfa_np = nfc[:, p128 % 8].T                       # [128, V]
        nfb_np = nfc[:, 8 + p128 % 8].T
        # residual pair idx (vloc >> 1), wrapped per 16-partition group:
        # slot j -> group j//NRES_G, col (j%NRES_G)//16, part (j%NRES_G)%16
        rv = np.zeros(NRES, np.int64)
        rv[:nres] = vloc[resid] >> 1
        ridx_np = np.zeros((128, NRES_G // 16), np.int16)
        for g in range(8):
            blk = rv[g * NRES_G : (g + 1) * NRES_G].reshape(NRES_G // 16, 16)
            ridx_np[g * 16 : (g + 1) * 16, :] = blk.T
        ins.append({
            "efT": np.ascontiguousarray(ef_pad[lo : lo + V_CORE].T.astype(BF)),
            "nfa": np.ascontiguousarray(nfa_np.astype(BF)),
            "nfb": np.ascontiguousarray(nfb_np.astype(BF)),
            "ridx": np.ascontiguousarray(ridx_np),
            **shared,
        })
        meta.append((vloc, rank, resid))

    r = bass_utils.run_bass_kernel_spmd(
        _cache["fused"], ins, core_ids=list(range(N_CORES)), trace=TRACE)
    last_exec_ns["mlp"] = r.exec_time_ns
    last_exec_ns["gather"] = 0

    # ---- host unshard: bijective relayout of device-written slots ----
    out = np.empty((E, D_NODE), np.float32)
    frange = np.arange(D_NODE)
    for c in range(N_CORES):
        vloc, rank, resid = meta[c]
        rep = np.stack([np.asarray(r.results[c]["rep0"]).astype(np.float32),
                        np.asarray(r.results[c]["rep1"]).astype(np.float32)])
        res_c = np.asarray(r.results[c]["res"]).astype(np.float32)
        ec = len(vloc)
        vals = np.empty((ec, D_NODE), np.float32)
        main = ~resid
        rm = rank[main]
        vals[main] = rep[(rm // 8)[:, None],
                         (16 * (rm % 8))[:, None] + frange[None, :],
                         vloc[main][:, None]]
        j = np.arange(int(resid.sum()))
        vals[resid] = res_c[(16 * (j // NRES_G))[:, None] + frange[None, :],
                            (2 * (j % NRES_G) + (vloc[resid] & 1))[:, None]]
        out[order[bounds[c] : bounds[c + 1]]] = vals
    return out

